# revision 65
# baseline (speedup 1.0000x reference)
"""GAT (2-layer, PyG-style) Trainium2 kernel — 8-core SPMD, v2.

Sharding: nodes greedy-balanced by degree into (cores*T) tiles of 128 slots.
Per core: fp16 GEMM [x_T @ (W1|W1@A1)] -> packed node rows (h||al_s||al_d)
-> chunked AllGather (overlapped with GEMM1) -> per-edge dma_gather of src
rows (2 calls per dst tile; self-loops come via one direct DMA) ->
host-precomputed one-hot matrices (oh / ohT) DMA'd per tile drive the
segment-softmax + scatter-add on the PE -> normalize, ELU, PE-transpose,
GEMM2 inline per tile -> chunked AllGather (overlapped with L1 edge phase)
-> L2 edge pipeline folds exp(z) into the one-hot mask (one 257-col matmul
per chunk; denominator via a ones-column in the row) -> output.

Self-contained: only numpy + the in-container concourse stack.
"""

import heapq
import os
import sys

import numpy as np

sys.path.insert(0, "/opt/trn_rl_repo")

import concourse.bacc as bacc  # noqa: E402
import concourse.bass as bass  # noqa: E402,F401
import concourse.mybir as mybir  # noqa: E402
import concourse.tile as tile  # noqa: E402

F16 = mybir.dt.float16
F32 = mybir.dt.float32
F8 = mybir.dt.float8e4
I16 = mybir.dt.int16

NEG = 0.2          # leaky relu slope
SHIFT1 = 3.0       # exp shift, layer 1 (softmax-invariant, keeps fp16 happy)
SHIFT2 = 1.0       # exp shift, layer 2
EPS = 1e-12

# Layer dims (fixed by the problem)
D_IN, H1, C1 = 1024, 8, 64
D_H = H1 * C1            # 512
D_OUT = 256
ROW1 = 640               # fp16 slots per L1 node row: 512 h + 32 al(f32) + pad
ROW2 = 384               # fp16 slots per L2 node row: 256 h + 1 + 32.. pad
AL1_OFF = 256            # f32-view col of al_s in L1 row (f16 col 512)
ALD1_F16 = 528           # f16 col of al_d (f32 pair view) in L1 row
AL2_F16 = 258            # f16 col of al_s(f32) in L2 row
K_G = 16                 # gathered chunks per tile (plus 1 self chunk)
K_CH = K_G + 1           # total chunks per tile


# ------------------------------------------------------------------ host prep

def _balance_nodes(dst, n_nodes, n_tiles):
    """Greedy-balance nodes into n_tiles tiles of <=128 slots by in-degree.
    Returns slot_of_node [n_nodes] (slot = tile*128 + row)."""
    deg = np.bincount(dst, minlength=n_nodes)
    order = np.argsort(-deg, kind="stable")
    heap = [(0, t) for t in range(n_tiles)]
    heapq.heapify(heap)
    fill = np.zeros(n_tiles, np.int64)
    slot = np.empty(n_nodes, np.int64)
    for n in order:
        load, t = heapq.heappop(heap)
        slot[n] = t * 128 + fill[t]
        fill[t] += 1
        if fill[t] < 128:
            heapq.heappush(heap, (load + int(deg[n]), t))
    return slot


def _wrap_idx(gidx):
    """int16 idx array in dma_gather layout [128, n/16]: col s holds idx
    [16s..16s+15] in partitions 0..15, replicated to 128."""
    n = len(gidx)
    assert n % 16 == 0
    w = gidx.astype(np.int16).reshape(n // 16, 16).T  # [16, n/16]
    return np.ascontiguousarray(np.tile(w, (8, 1)))


def prep(x, edge_index, W1, a1_src, a1_dst, W2, a2_src, a2_dst, cores, ag_g):
    n_nodes = x.shape[0]
    t_per_core = int(np.ceil(n_nodes / (cores * 128)))
    n_tiles = cores * t_per_core
    s_core = t_per_core * 128          # slots per core
    T = t_per_core

    src = np.asarray(edge_index[0]).astype(np.int64)
    dst = np.asarray(edge_index[1]).astype(np.int64)
    # self-loops handled as the identity chunk 0, not in the gather stream
    dst_all = np.concatenate([dst, np.arange(n_nodes)])
    slot = _balance_nodes(dst_all, n_nodes, n_tiles)
    node_of_slot = np.full(n_tiles * 128, -1, np.int64)
    node_of_slot[slot] = np.arange(n_nodes)
    valid_slot = node_of_slot >= 0

    # h_full global row of an admin slot (AG-chunk-major; ag_g is the list
    # of AG chunk sizes in tiles — uneven chunks let the first collective
    # fire early under GEMM1. The same layout serves both layers.)
    tt = np.arange(n_tiles * 128) // 128
    core_of = tt // T
    t_of = tt % T
    r_of = np.arange(n_tiles * 128) % 128
    h2row_of_slot = core_of * s_core + t_of * 128 + r_of
    if ag_g:
        assert sum(ag_g) == T
        chunk_of_t = np.repeat(np.arange(len(ag_g)), ag_g)
        start_of_chunk = np.concatenate([[0], np.cumsum(ag_g)])
        c_of = chunk_of_t[t_of]
        hrow_of_slot = (start_of_chunk[c_of] * cores * 128
                        + core_of * (np.asarray(ag_g)[c_of] * 128)
                        + (t_of - start_of_chunk[c_of]) * 128 + r_of)
    else:
        hrow_of_slot = h2row_of_slot

    # group non-self edges by dst tile
    e_tile = slot[dst] // 128
    order = np.argsort(e_tile, kind="stable")
    s_sorted = slot[src][order]
    d_sorted = slot[dst][order]
    counts = np.bincount(e_tile, minlength=n_tiles)
    assert counts.max() <= K_G * 128, counts.max()
    cap = K_G * 128
    starts = np.concatenate([[0], np.cumsum(counts)])

    gidx = np.zeros(n_tiles * cap, np.int64)        # h1_full row per gathered edge
    gidx2 = np.zeros(n_tiles * cap, np.int64)       # h2_full row (core-major)
    # one-hot constants: oh[p, (t*K_CH+j)*128 + d], ohT transposed
    oh = np.zeros((cores, 128, T * K_CH * 128), np.float16)
    ohT = np.zeros((cores, 128, T * K_CH * 128), np.float16)
    for g_tile in range(n_tiles):
        a, b = starts[g_tile], starts[g_tile + 1]
        c, t = g_tile // T, g_tile % T
        gidx[g_tile * cap: g_tile * cap + (b - a)] = hrow_of_slot[s_sorted[a:b]]
        gidx2[g_tile * cap: g_tile * cap + (b - a)] = h2row_of_slot[s_sorted[a:b]]
        ncol = (t * K_CH) * 128
        p = np.arange(b - a)
        j = 1 + p // 128
        pp = p % 128
        d = (d_sorted[a:b] % 128).astype(np.int64)
        oh[c, pp, ncol + j * 128 + d] = 1.0
        ohT[c, d, ncol + j * 128 + pp] = 1.0
        # chunk 0: masked identity for the self-loops
        rr = np.arange(128)
        m = valid_slot[g_tile * 128 + rr]
        oh[c, rr[m], ncol + rr[m]] = 1.0
        ohT[c, rr[m], ncol + rr[m]] = 1.0

    per_idx, per_idx2 = [], []
    for c in range(cores):
        lo, hi = c * T * cap, (c + 1) * T * cap
        per_idx.append(_wrap_idx(gidx[lo:hi]))
        per_idx2.append(_wrap_idx(gidx2[lo:hi]))

    A1 = np.zeros((D_H, 2 * H1), np.float32)
    for h in range(H1):
        A1[h * C1: (h + 1) * C1, h] = np.asarray(a1_src)[h]
        A1[h * C1: (h + 1) * C1, H1 + h] = np.asarray(a1_dst)[h]
    W1e = np.concatenate([np.asarray(W1), np.asarray(W1) @ A1], 1)  # [1024,528]
    A2 = np.stack([np.asarray(a2_src)[0], np.asarray(a2_dst)[0]], 1)  # [256,2]
    W2e = np.concatenate([np.asarray(W2), np.asarray(W2) @ A2], 1)    # [512,258]

    x16 = np.asarray(x).astype(np.float16)
    xT = []
    for c in range(cores):
        sl = node_of_slot[c * s_core: (c + 1) * s_core]
        xs = np.zeros((s_core, D_IN), np.float16)
        ok = sl >= 0
        xs[ok] = x16[sl[ok]]
        xT.append(np.ascontiguousarray(xs.T))

    cfg = dict(cores=cores, t_per_core=T, s_core=s_core, ag_g=ag_g)
    host = dict(
        W1e=W1e.astype(np.float16), W2e=W2e.astype(np.float16),
        xT=xT, idx=per_idx, idx2=per_idx2,
        oh=[np.ascontiguousarray(oh[c]) for c in range(cores)],
        ohT=[np.ascontiguousarray(ohT[c]) for c in range(cores)],
        ident=np.eye(128, dtype=np.float16),
    )
    return cfg, host, dict(slot=slot, s_core=s_core)


# -------------------------------------------------------------- device program

def build(cfg):
    cores, T = cfg["cores"], cfg["t_per_core"]
    S = cfg["s_core"]                      # slots per core
    SG = S * cores                         # global slots
    G = cfg["ag_g"]                        # tiles per AG chunk (0 = one-shot)
    NIDX = T * K_G * 128
    KIN = D_IN // 128

    nc = bacc.Bacc("TRN2", target_bir_lowering=False, debug=False,
                   num_devices=cores)
    xT_d = nc.dram_tensor("xT", [D_IN, S], F16, kind="ExternalInput")
    W1e_d = nc.dram_tensor("W1e", [D_IN, D_H + 2 * H1], F16, kind="ExternalInput")
    W2e_d = nc.dram_tensor("W2e", [D_H, D_OUT + 2], F16, kind="ExternalInput")
    idx_d = nc.dram_tensor("idx", [128, NIDX // 16], I16, kind="ExternalInput")
    idx2_d = nc.dram_tensor("idx2", [128, NIDX // 16], I16, kind="ExternalInput")
    oh_d = nc.dram_tensor("oh", [128, T * K_CH * 128], F16, kind="ExternalInput")
    ohT_d = nc.dram_tensor("ohT", [128, T * K_CH * 128], F16, kind="ExternalInput")
    id_d = nc.dram_tensor("ident", [128, 128], F16, kind="ExternalInput")
    out_d = nc.dram_tensor("out", [S, D_OUT], F32, kind="ExternalOutput")

    ag1_in = nc.dram_tensor("ag1_in", [S, ROW1], F16)
    ag2_in = nc.dram_tensor("ag2_in", [S, ROW2], F16)
    ald2_d = nc.dram_tensor("ald2", [T, 128], F32)
    assert cores > 1
    h1_full = nc.dram_tensor("h1_full", [SG, ROW1], F16, addr_space="Shared")
    h2_full = nc.dram_tensor("h2_full", [SG, ROW2], F16, addr_space="Shared")

    AOT = mybir.AluOpType
    AFT = mybir.ActivationFunctionType
    rg = [list(range(cores))]

    def loc1_rows(t):
        return ag1_in[t * 128:(t + 1) * 128, :]

    starts = [0]
    for n in (G or []):
        starts.append(starts[-1] + n)

    def ag_chunk(layer, c):
        """AllGather AG chunk c (tiles starts[c]..starts[c+1]-1), layer 1/2."""
        src_t, dst_t = (ag1_in, h1_full) if layer == 1 else (ag2_in, h2_full)
        if G:
            a, b = starts[c] * 128, starts[c + 1] * 128
            off = starts[c] * cores * 128
            nc.gpsimd.collective_compute(
                "AllGather", AOT.bypass, replica_groups=rg,
                ins=[src_t[a:b, :].opt()],
                outs=[dst_t[off:off + cores * (b - a), :].opt()])
        else:
            nc.gpsimd.collective_compute(
                "AllGather", AOT.bypass, replica_groups=rg,
                ins=[src_t[:].opt()], outs=[dst_t[:].opt()])

    ag_fire = {starts[c + 1] - 1: c for c in range(len(G))} if G else {}

    with tile.TileContext(nc) as tc:
      with tc.tile_pool(name="const", bufs=1) as constp:
        ident_t = constp.tile([128, 128], F16)
        idx_t = constp.tile([128, NIDX // 16], I16)
        idx2_t = constp.tile([128, NIDX // 16], I16)
        W2e_t = constp.tile([128, 4, D_OUT + 2], F16)
        ones1_t = constp.tile([1, 128], F16)
        shift1_t = constp.tile([128, 1], F32)
        shift2_t = constp.tile([128, 1], F32)
        nc.sync.dma_start(ident_t[:], id_d[:])
        nc.sync.dma_start(idx_t[:], idx_d[:])
        nc.sync.dma_start(idx2_t[:], idx2_d[:])
        nc.sync.dma_start(W2e_t[:], W2e_d[:].rearrange("(k p) n -> p k n", p=128))
        nc.vector.memset(ones1_t[:], 1.0)
        nc.vector.memset(shift1_t[:], -SHIFT1)
        nc.vector.memset(shift2_t[:], -SHIFT2)

        # ---------------- GEMM1: h||al = xT.T @ W1e ----------------
        with (
            tc.tile_pool(name="gemm1", bufs=1) as g1p,
            tc.tile_pool(name="g1ps", bufs=2, space="PSUM") as g1ps,
            tc.tile_pool(name="rowio", bufs=3) as rowp,
        ):
            xT_t = g1p.tile([128, KIN, S], F16)
            W1e_t = g1p.tile([128, KIN, D_H + 2 * H1], F16)
            nc.sync.dma_start(xT_t[:], xT_d[:].rearrange("(k p) n -> p k n", p=128))
            nc.sync.dma_start(W1e_t[:], W1e_d[:].rearrange("(k p) n -> p k n", p=128))
            for t in range(T):
                ph = g1ps.tile([128, D_H], F32, tag="g1h")
                pa = g1ps.tile([128, 2 * H1], F32, tag="g1a")
                for k in range(KIN):
                    lw = xT_t[:, k, t * 128:(t + 1) * 128]
                    nc.tensor.matmul(ph[:], lw, W1e_t[:, k, 0:D_H],
                                     start=(k == 0), stop=(k == KIN - 1))
                    nc.tensor.matmul(pa[:], lw, W1e_t[:, k, D_H:],
                                     start=(k == 0), stop=(k == KIN - 1))
                row = rowp.tile([128, ROW1], F16, tag="r1")
                nc.scalar.copy(row[:, 0:D_H], ph[:])
                nc.scalar.copy(row[:, D_H:D_H + 32].bitcast(F32), pa[:])
                nc.sync.dma_start(
                    loc1_rows(t)[:, 0:D_H + 32], row[:, 0:D_H + 32])
                if t in ag_fire:
                    ag_chunk(1, ag_fire[t])

        if not G:
            ag_chunk(1, 0)

        # ---------------- L1 edge phase ----------------
        def l1_phase():
            with (
                tc.tile_pool(name="gath", bufs=4) as gp,
                tc.tile_pool(name="ohp", bufs=3) as ohp,
                tc.tile_pool(name="ework", bufs=3) as wp,
                tc.tile_pool(name="eps", bufs=2, space="PSUM") as pp,
                tc.tile_pool(name="epsald", bufs=2, space="PSUM") as pp3,
                tc.tile_pool(name="epost", bufs=1, space="PSUM") as pp2,
                tc.tile_pool(name="outw", bufs=3) as op,
            ):
                for t in range(T):
                    g = gp.tile([128, K_CH, ROW1], F16, tag="g")
                    nc.sync.dma_start(g[:, 0, :], loc1_rows(t))
                    for c0 in (1, 9):
                        nc.gpsimd.dma_gather(
                            g[:, c0:c0 + 8, :], h1_full[:],
                            idx_t[:, (t * K_G + c0 - 1) * 8:
                                  (t * K_G + c0 + 7) * 8],
                            1024, 1024, ROW1)
                    oh_t = ohp.tile([128, K_CH, 128], F16, tag="oh1")
                    ohT_t = ohp.tile([128, K_CH, 128], F16, tag="ohT1")
                    nc.sync.dma_start(
                        oh_t[:], oh_d[:, t * K_CH * 128:(t + 1) * K_CH * 128]
                        .rearrange("p (j d) -> p j d", d=128))
                    nc.sync.dma_start(
                        ohT_t[:], ohT_d[:, t * K_CH * 128:(t + 1) * K_CH * 128]
                        .rearrange("p (j d) -> p j d", d=128))
                    ald_f = wp.tile([128, 2 * H1], F16, tag="aldf")
                    nc.sync.dma_start(
                        ald_f[:],
                        loc1_rows(t)[:, ALD1_F16:ALD1_F16 + 2 * H1])
                    ald16 = wp.tile([128, H1], F16, tag="ald16")
                    nc.vector.tensor_copy(ald16[:], ald_f[:].bitcast(F32))
                    psA = pp.tile([128, D_H], F32, tag="eA")
                    psB = pp.tile([128, H1], F32, tag="eB")
                    for jj in range(0, K_CH, 2):
                        np2 = min(2, K_CH - jj)
                        al = g[:, jj:jj + np2,
                               2 * AL1_OFF:2 * AL1_OFF + 2 * H1].bitcast(F32)
                        psAld = pp3.tile([128, 2, H1], F32, tag="ald")
                        for j in range(np2):
                            nc.tensor.matmul(
                                psAld[:, j, :], ohT_t[:, jj + j, :],
                                ald16[:], start=True, stop=True)
                        z = wp.tile([128, 2, H1], F32, tag="z",
                                    name="z")[:, 0:np2, :]
                        p16 = wp.tile([128, 2, H1], F16, tag="p",
                                      name="p")[:, 0:np2, :]
                        V = wp.tile([128, 2, D_H], F16, tag="v",
                                    name="v")[:, 0:np2, :]
                        nc.vector.tensor_tensor(
                            z, al, psAld[:, 0:np2, :], op=AOT.add)
                        nc.vector.scalar_tensor_tensor(
                            z, z, NEG, z, op0=AOT.mult, op1=AOT.max)
                        nc.scalar.activation(p16, z, AFT.Exp, bias=shift1_t[:])
                        nc.vector.tensor_tensor(
                            V.rearrange("p j (h c) -> p j h c", h=H1),
                            g[:, jj:jj + np2, 0:D_H]
                            .rearrange("p j (h c) -> p j h c", h=H1),
                            p16.to_broadcast([128, np2, H1, C1]),
                            op=AOT.mult)
                        for j in range(np2):
                            st = (jj + j == 0)
                            sp = (jj + j == K_CH - 1)
                            nc.tensor.matmul(psA[:], oh_t[:, jj + j, :],
                                             V[:, j, :], start=st, stop=sp)
                            nc.tensor.matmul(psB[:], oh_t[:, jj + j, :],
                                             p16[:, j, :], start=st, stop=sp)
                    l1_consume(t, psA, psB, wp, op, pp2)

        # ---------------- L1 consumer: normalize, ELU, GEMM2, row2 ----------
        def l1_consume(t, psA, psB, wp, op, pp2):
            s_sb = wp.tile([128, H1], F32, tag="s1")
            rinv = wp.tile([128, H1], F32, tag="ri1")
            nc.scalar.activation(s_sb[:], psB[:], AFT.Copy, bias=EPS)
            nc.vector.reciprocal(rinv[:], s_sb[:])
            h1n = wp.tile([128, D_H], F16, tag="h1n")
            for h in range(H1):
                nc.scalar.activation(
                    h1n[:, h * C1:(h + 1) * C1], psA[:, h * C1:(h + 1) * C1],
                    AFT.Copy, scale=rinv[:, h:h + 1])
            r1 = wp.tile([128, D_H], F16, tag="relu")
            m1 = wp.tile([128, D_H], F16, tag="mrelu")
            e1 = wp.tile([128, D_H], F16, tag="eneg")
            h1e = wp.tile([128, D_H], F16, tag="h1e")
            nc.scalar.activation(r1[:], h1n[:], AFT.Relu)
            nc.scalar.activation(m1[:], h1n[:], AFT.Relu, scale=-1.0)
            nc.scalar.activation(e1[:], m1[:], AFT.Exp, scale=-1.0)
            nc.vector.scalar_tensor_tensor(
                h1e[:], e1[:], -1.0, r1[:], op0=AOT.add, op1=AOT.add)
            psT = pp2.tile([128, 4, 128], F16, tag="psT")
            h1eT = wp.tile([128, 4, 128], F16, tag="h1eT")
            for k in range(4):
                nc.tensor.transpose(
                    psT[:, k, :], h1e[:, k * 128:(k + 1) * 128], ident_t[:])
                nc.scalar.copy(h1eT[:, k, :], psT[:, k, :])
            ps2 = pp2.tile([128, D_OUT + 2], F32, tag="ps2")
            for k in range(4):
                nc.tensor.matmul(ps2[:], h1eT[:, k, :], W2e_t[:, k, :],
                                 start=(k == 0), stop=(k == 3))
            row2 = op.tile([128, ROW2], F16, tag="r2")
            nc.scalar.copy(row2[:, 0:D_OUT], ps2[:, 0:D_OUT])
            nc.vector.memset(row2[:, D_OUT:D_OUT + 1], 1.0)
            nc.scalar.copy(row2[:, AL2_F16:AL2_F16 + 4].bitcast(F32),
                           ps2[:, D_OUT:D_OUT + 2])
            nc.sync.dma_start(
                ag2_in[t * 128:(t + 1) * 128, 0:AL2_F16 + 4],
                row2[:, 0:AL2_F16 + 4])
            # ald2 of the tile's rows, laid out along the free dim for the
            # L2 aldT broadcast
            ald2c = op.tile([128, 1], F32, tag="ald2c")
            nc.scalar.copy(ald2c[:], ps2[:, D_OUT + 1:D_OUT + 2])
            nc.sync.dma_start(ald2_d[t:t + 1, :].rearrange("a b -> b a"),
                              ald2c[:])

        l1_phase()

        # AG2 one-shot: h2_full is core-major (idx2), and a single
        # collective moves the same bytes faster than 4 row-sliced ones
        nc.gpsimd.collective_compute(
            "AllGather", AOT.bypass, replica_groups=rg,
            ins=[ag2_in[:].opt()], outs=[h2_full[:].opt()])

        # ---------------- L2 edge phase: ohalpha scheme ----------
        def l2_phase():
            with (
                tc.tile_pool(name="gath2", bufs=3) as gp,
                tc.tile_pool(name="ohp2", bufs=3) as ohp,
                tc.tile_pool(name="ework2", bufs=3) as wp,
                tc.tile_pool(name="eps2", bufs=2, space="PSUM") as pp,
                tc.tile_pool(name="epsbc", bufs=2, space="PSUM") as ppb,
                tc.tile_pool(name="outw2", bufs=3) as op,
            ):
                for t in range(T):
                    g = gp.tile([128, K_CH, ROW2], F16, tag="g2")
                    nc.sync.dma_start(
                        g[:, 0, :], ag2_in[t * 128:(t + 1) * 128, :])
                    for c0 in (1, 9):
                        nc.gpsimd.dma_gather(
                            g[:, c0:c0 + 8, :], h2_full[:],
                            idx2_t[:, (t * K_G + c0 - 1) * 8:
                                   (t * K_G + c0 + 7) * 8],
                            1024, 1024, ROW2)
                    oh_t = ohp.tile([128, K_CH, 128], F16, tag="oh2")
                    nc.sync.dma_start(
                        oh_t[:], oh_d[:, t * K_CH * 128:(t + 1) * K_CH * 128]
                        .rearrange("p (j d) -> p j d", d=128))
                    aldr = wp.tile([1, 128], F32, tag="aldr")
                    aldr16 = wp.tile([1, 128], F16, tag="aldr16")
                    nc.sync.dma_start(aldr[:], ald2_d[t:t + 1, :])
                    nc.vector.tensor_copy(aldr16[:], aldr[:])
                    psAldT = ppb.tile([128, 128], F32, tag="aldT")
                    nc.tensor.matmul(psAldT[:], ones1_t[:], aldr16[:],
                                     start=True, stop=True)
                    psAB = pp.tile([128, D_OUT + 1], F32, tag="eAB")
                    for j in range(K_CH):
                        alj = g[:, j, AL2_F16:AL2_F16 + 2].bitcast(F32)
                        z = wp.tile([128, 128], F16, tag="z2")
                        nc.vector.tensor_scalar(
                            z[:], psAldT[:], alj, None, op0=AOT.add)
                        nc.vector.scalar_tensor_tensor(
                            z[:], z[:], NEG, z[:], op0=AOT.mult, op1=AOT.max)
                        pc = wp.tile([128, 128], F16, tag="pc")
                        nc.scalar.activation(pc[:], z[:], AFT.Exp,
                                             bias=shift2_t[:])
                        oha = wp.tile([128, 128], F16, tag="oha")
                        nc.vector.tensor_tensor(
                            oha[:], pc[:], oh_t[:, j, :], op=AOT.mult)
                        nc.tensor.matmul(psAB[:], oha[:],
                                         g[:, j, 0:D_OUT + 1],
                                         start=(j == 0), stop=(j == K_CH - 1))
                    s_sb = wp.tile([128, 1], F32, tag="s2")
                    rinv = wp.tile([128, 1], F32, tag="ri2")
                    nc.scalar.activation(s_sb[:], psAB[:, D_OUT:D_OUT + 1],
                                         AFT.Copy, bias=EPS)
                    nc.vector.reciprocal(rinv[:], s_sb[:])
                    o = op.tile([128, D_OUT], F32, tag="of")
                    nc.scalar.activation(o[:], psAB[:, 0:D_OUT], AFT.Copy,
                                         scale=rinv[:])
                    nc.sync.dma_start(out_d[t * 128:(t + 1) * 128, :], o[:])

        l2_phase()

    nc.compile()
    return nc


# ---------------------------------------------------------------- entry point

def make_in_maps(cfg, host):
    return [{
        "xT": host["xT"][c], "W1e": host["W1e"], "W2e": host["W2e"],
        "idx": host["idx"][c], "idx2": host["idx2"][c],
        "oh": host["oh"][c], "ohT": host["ohT"][c],
        "ident": host["ident"],
    } for c in range(cfg["cores"])]


def kernel(x, edge_index, W1, a1_src, a1_dst, b1, W2, a2_src, a2_dst, b2,
           cores=8, runner=None):
    x = np.asarray(x)
    edge_index = np.asarray(edge_index)
    assert np.allclose(np.asarray(b1), 0.0), "nonzero b1 unsupported"
    agspec = os.environ.get("GAT_AGCH", "6,8,8,8")
    ag_g = [int(v) for v in agspec.split(",") if v] if agspec != "0" else 0
    cfg, host, post = prep(x, edge_index, W1, a1_src, a1_dst,
                           W2, a2_src, a2_dst, cores, ag_g)
    nc = build(cfg)
    in_maps = make_in_maps(cfg, host)
    if runner is None:
        from concourse.bass_utils import run_bass_kernel_spmd
        res = run_bass_kernel_spmd(nc, in_maps, core_ids=list(range(cores)),
                                   trace=os.environ.get("GAT_TRACE", "") == "1")
        outs = [r["out"] for r in res.results]
        kernel.last_exec_ns = res.exec_time_ns
    else:
        outs = runner(nc, in_maps)
    slots = np.concatenate(outs, 0)          # [cores*S, D_OUT]
    full = slots[post["slot"]]               # back to node order
    return (full + np.asarray(b2)[None, :]).astype(np.float32)


kernel.last_exec_ns = None


# revision 67
# speedup vs baseline: 1.0466x; 1.0466x over previous
"""GAT (2-layer, PyG-style) Trainium2 kernel — 8-core SPMD, v2.

Sharding: nodes greedy-balanced by degree into (cores*T) tiles of 128 slots.
Per core: fp16 GEMM [x_T @ (W1|W1@A1)] -> packed node rows (h||al_s||al_d)
-> chunked AllGather (overlapped with GEMM1) -> per-edge dma_gather of src
rows (2 calls per dst tile; self-loops come via one direct DMA) ->
host-precomputed one-hot matrices (oh / ohT) DMA'd per tile drive the
segment-softmax + scatter-add on the PE -> normalize, ELU, PE-transpose,
GEMM2 inline per tile -> chunked AllGather (overlapped with L1 edge phase)
-> L2 edge pipeline folds exp(z) into the one-hot mask (one 257-col matmul
per chunk; denominator via a ones-column in the row) -> output.

Self-contained: only numpy + the in-container concourse stack.
"""

import heapq
import os
import sys

import numpy as np

sys.path.insert(0, "/opt/trn_rl_repo")

import concourse.bacc as bacc  # noqa: E402
import concourse.bass as bass  # noqa: E402,F401
import concourse.mybir as mybir  # noqa: E402
import concourse.tile as tile  # noqa: E402

F16 = mybir.dt.float16
F32 = mybir.dt.float32
F8 = mybir.dt.float8e4
I16 = mybir.dt.int16

NEG = 0.2          # leaky relu slope
SHIFT1 = 3.0       # exp shift, layer 1 (softmax-invariant, keeps fp16 happy)
SHIFT2 = 1.0       # exp shift, layer 2
EPS = 1e-12

# Layer dims (fixed by the problem)
D_IN, H1, C1 = 1024, 8, 64
D_H = H1 * C1            # 512
D_OUT = 256
ROW1 = 640               # fp16 slots per L1 node row: 512 h + 32 al(f32) + pad
ROW2 = 384               # fp16 slots per L2 node row: 256 h + 1 + 32.. pad
AL1_OFF = 256            # f32-view col of al_s in L1 row (f16 col 512)
ALD1_F16 = 528           # f16 col of al_d (f32 pair view) in L1 row
AL2_F16 = 258            # f16 col of al_s(f32) in L2 row
K_G = 16                 # gathered chunks per tile (plus 1 self chunk)
K_CH = K_G + 1           # total chunks per tile


# ------------------------------------------------------------------ host prep

def _balance_nodes(dst, n_nodes, n_tiles):
    """Greedy-balance nodes into n_tiles tiles of <=128 slots by in-degree.
    Returns slot_of_node [n_nodes] (slot = tile*128 + row)."""
    deg = np.bincount(dst, minlength=n_nodes)
    order = np.argsort(-deg, kind="stable")
    heap = [(0, t) for t in range(n_tiles)]
    heapq.heapify(heap)
    fill = np.zeros(n_tiles, np.int64)
    slot = np.empty(n_nodes, np.int64)
    for n in order:
        load, t = heapq.heappop(heap)
        slot[n] = t * 128 + fill[t]
        fill[t] += 1
        if fill[t] < 128:
            heapq.heappush(heap, (load + int(deg[n]), t))
    return slot


def _wrap_idx(gidx):
    """int16 idx array in dma_gather layout [128, n/16]: col s holds idx
    [16s..16s+15] in partitions 0..15, replicated to 128."""
    n = len(gidx)
    assert n % 16 == 0
    w = gidx.astype(np.int16).reshape(n // 16, 16).T  # [16, n/16]
    return np.ascontiguousarray(np.tile(w, (8, 1)))


def prep(x, edge_index, W1, a1_src, a1_dst, W2, a2_src, a2_dst, cores, ag_g):
    n_nodes = x.shape[0]
    t_per_core = int(np.ceil(n_nodes / (cores * 128)))
    n_tiles = cores * t_per_core
    s_core = t_per_core * 128          # slots per core
    T = t_per_core

    src = np.asarray(edge_index[0]).astype(np.int64)
    dst = np.asarray(edge_index[1]).astype(np.int64)
    # self-loops handled as the identity chunk 0, not in the gather stream
    dst_all = np.concatenate([dst, np.arange(n_nodes)])
    slot = _balance_nodes(dst_all, n_nodes, n_tiles)
    node_of_slot = np.full(n_tiles * 128, -1, np.int64)
    node_of_slot[slot] = np.arange(n_nodes)
    valid_slot = node_of_slot >= 0

    # h_full global row of an admin slot (AG-chunk-major; ag_g is the list
    # of AG chunk sizes in tiles — uneven chunks let the first collective
    # fire early under GEMM1. The same layout serves both layers.)
    tt = np.arange(n_tiles * 128) // 128
    core_of = tt // T
    t_of = tt % T
    r_of = np.arange(n_tiles * 128) % 128
    if ag_g:
        assert sum(ag_g) == T
        chunk_of_t = np.repeat(np.arange(len(ag_g)), ag_g)
        start_of_chunk = np.concatenate([[0], np.cumsum(ag_g)])
        c_of = chunk_of_t[t_of]
        hrow_of_slot = (start_of_chunk[c_of] * cores * 128
                        + core_of * (np.asarray(ag_g)[c_of] * 128)
                        + (t_of - start_of_chunk[c_of]) * 128 + r_of)
    else:
        hrow_of_slot = core_of * s_core + t_of * 128 + r_of

    # group non-self edges by dst tile; within a tile order by ascending
    # gather row for DRAM locality of the gather descriptors
    e_tile = slot[dst] // 128
    key = e_tile * (64 * 1024) + hrow_of_slot[slot[src]]
    order = np.argsort(key, kind="stable")
    s_sorted = slot[src][order]
    d_sorted = slot[dst][order]
    counts = np.bincount(e_tile, minlength=n_tiles)
    assert counts.max() <= K_G * 128, counts.max()
    cap = K_G * 128
    starts = np.concatenate([[0], np.cumsum(counts)])

    gidx = np.zeros(n_tiles * cap, np.int64)        # h_full row per gathered edge
    # one-hot constants: oh[p, (t*K_CH+j)*128 + d], ohT transposed
    oh = np.zeros((cores, 128, T * K_CH * 128), np.float16)
    ohT = np.zeros((cores, 128, T * K_CH * 128), np.float16)
    for g_tile in range(n_tiles):
        a, b = starts[g_tile], starts[g_tile + 1]
        c, t = g_tile // T, g_tile % T
        gidx[g_tile * cap: g_tile * cap + (b - a)] = hrow_of_slot[s_sorted[a:b]]
        ncol = (t * K_CH) * 128
        p = np.arange(b - a)
        j = 1 + p // 128
        pp = p % 128
        d = (d_sorted[a:b] % 128).astype(np.int64)
        oh[c, pp, ncol + j * 128 + d] = 1.0
        ohT[c, d, ncol + j * 128 + pp] = 1.0
        # chunk 0: masked identity for the self-loops
        rr = np.arange(128)
        m = valid_slot[g_tile * 128 + rr]
        oh[c, rr[m], ncol + rr[m]] = 1.0
        ohT[c, rr[m], ncol + rr[m]] = 1.0

    per_idx = []
    for c in range(cores):
        lo, hi = c * T * cap, (c + 1) * T * cap
        per_idx.append(_wrap_idx(gidx[lo:hi]))

    A1 = np.zeros((D_H, 2 * H1), np.float32)
    for h in range(H1):
        A1[h * C1: (h + 1) * C1, h] = np.asarray(a1_src)[h]
        A1[h * C1: (h + 1) * C1, H1 + h] = np.asarray(a1_dst)[h]
    W1e = np.concatenate([np.asarray(W1), np.asarray(W1) @ A1], 1)  # [1024,528]
    A2 = np.stack([np.asarray(a2_src)[0], np.asarray(a2_dst)[0]], 1)  # [256,2]
    W2e = np.concatenate([np.asarray(W2), np.asarray(W2) @ A2], 1)    # [512,258]

    x16 = np.asarray(x).astype(np.float16)
    xT = []
    for c in range(cores):
        sl = node_of_slot[c * s_core: (c + 1) * s_core]
        xs = np.zeros((s_core, D_IN), np.float16)
        ok = sl >= 0
        xs[ok] = x16[sl[ok]]
        xT.append(np.ascontiguousarray(xs.T))

    cfg = dict(cores=cores, t_per_core=T, s_core=s_core, ag_g=ag_g)
    host = dict(
        W1e=W1e.astype(np.float16), W2e=W2e.astype(np.float16),
        xT=xT, idx=per_idx,
        oh=[np.ascontiguousarray(oh[c]) for c in range(cores)],
        ohT=[np.ascontiguousarray(ohT[c]) for c in range(cores)],
        ident=np.eye(128, dtype=np.float16),
    )
    return cfg, host, dict(slot=slot, s_core=s_core)


# -------------------------------------------------------------- device program

def build(cfg):
    cores, T = cfg["cores"], cfg["t_per_core"]
    S = cfg["s_core"]                      # slots per core
    SG = S * cores                         # global slots
    G = cfg["ag_g"]                        # tiles per AG chunk (0 = one-shot)
    NIDX = T * K_G * 128
    KIN = D_IN // 128

    nc = bacc.Bacc("TRN2", target_bir_lowering=False, debug=False,
                   num_devices=cores)
    xT_d = nc.dram_tensor("xT", [D_IN, S], F16, kind="ExternalInput")
    W1e_d = nc.dram_tensor("W1e", [D_IN, D_H + 2 * H1], F16, kind="ExternalInput")
    W2e_d = nc.dram_tensor("W2e", [D_H, D_OUT + 2], F16, kind="ExternalInput")
    idx_d = nc.dram_tensor("idx", [128, NIDX // 16], I16, kind="ExternalInput")
    oh_d = nc.dram_tensor("oh", [128, T * K_CH * 128], F16, kind="ExternalInput")
    ohT_d = nc.dram_tensor("ohT", [128, T * K_CH * 128], F16, kind="ExternalInput")
    id_d = nc.dram_tensor("ident", [128, 128], F16, kind="ExternalInput")
    out_d = nc.dram_tensor("out", [S, D_OUT], F32, kind="ExternalOutput")

    ag1_in = nc.dram_tensor("ag1_in", [S, ROW1], F16)
    ag2_in = nc.dram_tensor("ag2_in", [S, ROW2], F16)
    ald2_d = nc.dram_tensor("ald2", [T, 128], F32)
    assert cores > 1
    h1_full = nc.dram_tensor("h1_full", [SG, ROW1], F16, addr_space="Shared")
    h2_full = nc.dram_tensor("h2_full", [SG, ROW2], F16, addr_space="Shared")

    AOT = mybir.AluOpType
    AFT = mybir.ActivationFunctionType
    rg = [list(range(cores))]

    def loc1_rows(t):
        return ag1_in[t * 128:(t + 1) * 128, :]

    starts = [0]
    for n in (G or []):
        starts.append(starts[-1] + n)

    def ag_chunk(layer, c):
        """AllGather AG chunk c (tiles starts[c]..starts[c+1]-1), layer 1/2."""
        src_t, dst_t = (ag1_in, h1_full) if layer == 1 else (ag2_in, h2_full)
        if G:
            a, b = starts[c] * 128, starts[c + 1] * 128
            off = starts[c] * cores * 128
            nc.gpsimd.collective_compute(
                "AllGather", AOT.bypass, replica_groups=rg,
                ins=[src_t[a:b, :].opt()],
                outs=[dst_t[off:off + cores * (b - a), :].opt()])
        else:
            nc.gpsimd.collective_compute(
                "AllGather", AOT.bypass, replica_groups=rg,
                ins=[src_t[:].opt()], outs=[dst_t[:].opt()])

    ag_fire = {starts[c + 1] - 1: c for c in range(len(G))} if G else {}

    with tile.TileContext(nc) as tc:
      with tc.tile_pool(name="const", bufs=1) as constp:
        ident_t = constp.tile([128, 128], F16)
        idx_t = constp.tile([128, NIDX // 16], I16)
        W2e_t = constp.tile([128, 4, D_OUT + 2], F16)
        ones1_t = constp.tile([1, 128], F16)
        shift1_t = constp.tile([128, 1], F32)
        shift2_t = constp.tile([128, 1], F32)
        nc.sync.dma_start(ident_t[:], id_d[:])
        nc.sync.dma_start(idx_t[:], idx_d[:])
        nc.sync.dma_start(W2e_t[:], W2e_d[:].rearrange("(k p) n -> p k n", p=128))
        nc.vector.memset(ones1_t[:], 1.0)
        nc.vector.memset(shift1_t[:], -SHIFT1)
        nc.vector.memset(shift2_t[:], -SHIFT2)

        # ---------------- GEMM1: h||al = xT.T @ W1e ----------------
        with (
            tc.tile_pool(name="gemm1", bufs=1) as g1p,
            tc.tile_pool(name="g1ps", bufs=2, space="PSUM") as g1ps,
            tc.tile_pool(name="rowio", bufs=3) as rowp,
        ):
            xT_t = g1p.tile([128, KIN, S], F16)
            W1e_t = g1p.tile([128, KIN, D_H + 2 * H1], F16)
            nc.sync.dma_start(xT_t[:], xT_d[:].rearrange("(k p) n -> p k n", p=128))
            nc.sync.dma_start(W1e_t[:], W1e_d[:].rearrange("(k p) n -> p k n", p=128))
            for t in range(T):
                ph = g1ps.tile([128, D_H], F32, tag="g1h")
                pa = g1ps.tile([128, 2 * H1], F32, tag="g1a")
                for k in range(KIN):
                    lw = xT_t[:, k, t * 128:(t + 1) * 128]
                    nc.tensor.matmul(ph[:], lw, W1e_t[:, k, 0:D_H],
                                     start=(k == 0), stop=(k == KIN - 1))
                    nc.tensor.matmul(pa[:], lw, W1e_t[:, k, D_H:],
                                     start=(k == 0), stop=(k == KIN - 1))
                row = rowp.tile([128, ROW1], F16, tag="r1")
                nc.scalar.copy(row[:, 0:D_H], ph[:])
                nc.scalar.copy(row[:, D_H:D_H + 32].bitcast(F32), pa[:])
                nc.sync.dma_start(
                    loc1_rows(t)[:, 0:D_H + 32], row[:, 0:D_H + 32])
                if t in ag_fire:
                    ag_chunk(1, ag_fire[t])

        if not G:
            ag_chunk(1, 0)

        # ---------------- L1 edge phase ----------------
        def l1_phase():
            with (
                tc.tile_pool(name="gath", bufs=4) as gp,
                tc.tile_pool(name="ohp", bufs=3) as ohp,
                tc.tile_pool(name="ework", bufs=3) as wp,
                tc.tile_pool(name="eps", bufs=2, space="PSUM") as pp,
                tc.tile_pool(name="epsald", bufs=2, space="PSUM") as pp3,
                tc.tile_pool(name="epost", bufs=1, space="PSUM") as pp2,
                tc.tile_pool(name="outw", bufs=3) as op,
            ):
                for t in range(T):
                    g = gp.tile([128, K_CH, ROW1], F16, tag="g")
                    nc.sync.dma_start(g[:, 0, :], loc1_rows(t))
                    for c0 in (1, 9):
                        nc.gpsimd.dma_gather(
                            g[:, c0:c0 + 8, :], h1_full[:],
                            idx_t[:, (t * K_G + c0 - 1) * 8:
                                  (t * K_G + c0 + 7) * 8],
                            1024, 1024, ROW1)
                    oh_t = ohp.tile([128, K_CH, 128], F16, tag="oh1")
                    ohT_t = ohp.tile([128, K_CH, 128], F16, tag="ohT1")
                    nc.sync.dma_start(
                        oh_t[:], oh_d[:, t * K_CH * 128:(t + 1) * K_CH * 128]
                        .rearrange("p (j d) -> p j d", d=128))
                    nc.sync.dma_start(
                        ohT_t[:], ohT_d[:, t * K_CH * 128:(t + 1) * K_CH * 128]
                        .rearrange("p (j d) -> p j d", d=128))
                    ald_f = wp.tile([128, 2 * H1], F16, tag="aldf")
                    nc.sync.dma_start(
                        ald_f[:],
                        loc1_rows(t)[:, ALD1_F16:ALD1_F16 + 2 * H1])
                    ald16 = wp.tile([128, H1], F16, tag="ald16")
                    nc.vector.tensor_copy(ald16[:], ald_f[:].bitcast(F32))
                    psA = pp.tile([128, D_H], F32, tag="eA")
                    psB = pp.tile([128, H1], F32, tag="eB")
                    for jj in range(0, K_CH, 2):
                        np2 = min(2, K_CH - jj)
                        al = g[:, jj:jj + np2,
                               2 * AL1_OFF:2 * AL1_OFF + 2 * H1].bitcast(F32)
                        psAld = pp3.tile([128, 2, H1], F32, tag="ald")
                        for j in range(np2):
                            nc.tensor.matmul(
                                psAld[:, j, :], ohT_t[:, jj + j, :],
                                ald16[:], start=True, stop=True)
                        z = wp.tile([128, 2, H1], F32, tag="z",
                                    name="z")[:, 0:np2, :]
                        p16 = wp.tile([128, 2, H1], F16, tag="p",
                                      name="p")[:, 0:np2, :]
                        V = wp.tile([128, 2, D_H], F16, tag="v",
                                    name="v")[:, 0:np2, :]
                        nc.vector.tensor_tensor(
                            z, al, psAld[:, 0:np2, :], op=AOT.add)
                        nc.vector.scalar_tensor_tensor(
                            z, z, NEG, z, op0=AOT.mult, op1=AOT.max)
                        nc.scalar.activation(p16, z, AFT.Exp, bias=shift1_t[:])
                        nc.vector.tensor_tensor(
                            V.rearrange("p j (h c) -> p j h c", h=H1),
                            g[:, jj:jj + np2, 0:D_H]
                            .rearrange("p j (h c) -> p j h c", h=H1),
                            p16.to_broadcast([128, np2, H1, C1]),
                            op=AOT.mult)
                        for j in range(np2):
                            st = (jj + j == 0)
                            sp = (jj + j == K_CH - 1)
                            nc.tensor.matmul(psA[:], oh_t[:, jj + j, :],
                                             V[:, j, :], start=st, stop=sp)
                            nc.tensor.matmul(psB[:], oh_t[:, jj + j, :],
                                             p16[:, j, :], start=st, stop=sp)
                    l1_consume(t, psA, psB, wp, op, pp2)
                    if t in ag_fire:
                        ag_chunk(2, ag_fire[t])

        # ---------------- L1 consumer: normalize, ELU, GEMM2, row2 ----------
        def l1_consume(t, psA, psB, wp, op, pp2):
            s_sb = wp.tile([128, H1], F32, tag="s1")
            rinv = wp.tile([128, H1], F32, tag="ri1")
            nc.scalar.activation(s_sb[:], psB[:], AFT.Copy, bias=EPS)
            nc.vector.reciprocal(rinv[:], s_sb[:])
            h1n = wp.tile([128, D_H], F16, tag="h1n")
            for h in range(H1):
                nc.scalar.activation(
                    h1n[:, h * C1:(h + 1) * C1], psA[:, h * C1:(h + 1) * C1],
                    AFT.Copy, scale=rinv[:, h:h + 1])
            r1 = wp.tile([128, D_H], F16, tag="relu")
            m1 = wp.tile([128, D_H], F16, tag="mrelu")
            e1 = wp.tile([128, D_H], F16, tag="eneg")
            h1e = wp.tile([128, D_H], F16, tag="h1e")
            nc.scalar.activation(r1[:], h1n[:], AFT.Relu)
            nc.scalar.activation(m1[:], h1n[:], AFT.Relu, scale=-1.0)
            nc.scalar.activation(e1[:], m1[:], AFT.Exp, scale=-1.0)
            nc.vector.scalar_tensor_tensor(
                h1e[:], e1[:], -1.0, r1[:], op0=AOT.add, op1=AOT.add)
            psT = pp2.tile([128, 4, 128], F16, tag="psT")
            h1eT = wp.tile([128, 4, 128], F16, tag="h1eT")
            for k in range(4):
                nc.tensor.transpose(
                    psT[:, k, :], h1e[:, k * 128:(k + 1) * 128], ident_t[:])
                nc.scalar.copy(h1eT[:, k, :], psT[:, k, :])
            ps2 = pp2.tile([128, D_OUT + 2], F32, tag="ps2")
            for k in range(4):
                nc.tensor.matmul(ps2[:], h1eT[:, k, :], W2e_t[:, k, :],
                                 start=(k == 0), stop=(k == 3))
            row2 = op.tile([128, ROW2], F16, tag="r2")
            nc.scalar.copy(row2[:, 0:D_OUT], ps2[:, 0:D_OUT])
            nc.vector.memset(row2[:, D_OUT:D_OUT + 1], 1.0)
            nc.scalar.copy(row2[:, AL2_F16:AL2_F16 + 4].bitcast(F32),
                           ps2[:, D_OUT:D_OUT + 2])
            nc.sync.dma_start(
                ag2_in[t * 128:(t + 1) * 128, 0:AL2_F16 + 4],
                row2[:, 0:AL2_F16 + 4])
            # ald2 of the tile's rows, laid out along the free dim for the
            # L2 aldT broadcast
            ald2c = op.tile([128, 1], F32, tag="ald2c")
            nc.scalar.copy(ald2c[:], ps2[:, D_OUT + 1:D_OUT + 2])
            nc.sync.dma_start(ald2_d[t:t + 1, :].rearrange("a b -> b a"),
                              ald2c[:])

        l1_phase()

        if not G:
            ag_chunk(2, 0)

        # ---------------- L2 edge phase: ohalpha scheme ----------
        def l2_phase():
            with (
                tc.tile_pool(name="gath2", bufs=3) as gp,
                tc.tile_pool(name="ohp2", bufs=3) as ohp,
                tc.tile_pool(name="ework2", bufs=3) as wp,
                tc.tile_pool(name="eps2", bufs=2, space="PSUM") as pp,
                tc.tile_pool(name="epsbc", bufs=2, space="PSUM") as ppb,
                tc.tile_pool(name="outw2", bufs=3) as op,
            ):
                for t in range(T):
                    g = gp.tile([128, K_CH, ROW2], F16, tag="g2")
                    nc.sync.dma_start(
                        g[:, 0, :], ag2_in[t * 128:(t + 1) * 128, :])
                    for c0 in (1, 9):
                        nc.gpsimd.dma_gather(
                            g[:, c0:c0 + 8, :], h2_full[:],
                            idx_t[:, (t * K_G + c0 - 1) * 8:
                                  (t * K_G + c0 + 7) * 8],
                            1024, 1024, ROW2)
                    oh_t = ohp.tile([128, K_CH, 128], F16, tag="oh2")
                    nc.sync.dma_start(
                        oh_t[:], oh_d[:, t * K_CH * 128:(t + 1) * K_CH * 128]
                        .rearrange("p (j d) -> p j d", d=128))
                    aldr = wp.tile([1, 128], F32, tag="aldr")
                    aldr16 = wp.tile([1, 128], F16, tag="aldr16")
                    nc.sync.dma_start(aldr[:], ald2_d[t:t + 1, :])
                    nc.vector.tensor_copy(aldr16[:], aldr[:])
                    psAldT = ppb.tile([128, 128], F32, tag="aldT")
                    nc.tensor.matmul(psAldT[:], ones1_t[:], aldr16[:],
                                     start=True, stop=True)
                    psAB = pp.tile([128, D_OUT + 1], F32, tag="eAB")
                    for j in range(K_CH):
                        alj = g[:, j, AL2_F16:AL2_F16 + 2].bitcast(F32)
                        z = wp.tile([128, 128], F16, tag="z2")
                        nc.vector.tensor_scalar(
                            z[:], psAldT[:], alj, None, op0=AOT.add)
                        nc.vector.scalar_tensor_tensor(
                            z[:], z[:], NEG, z[:], op0=AOT.mult, op1=AOT.max)
                        pc = wp.tile([128, 128], F16, tag="pc")
                        nc.scalar.activation(pc[:], z[:], AFT.Exp,
                                             bias=shift2_t[:])
                        oha = wp.tile([128, 128], F16, tag="oha")
                        nc.vector.tensor_tensor(
                            oha[:], pc[:], oh_t[:, j, :], op=AOT.mult)
                        nc.tensor.matmul(psAB[:], oha[:],
                                         g[:, j, 0:D_OUT + 1],
                                         start=(j == 0), stop=(j == K_CH - 1))
                    s_sb = wp.tile([128, 1], F32, tag="s2")
                    rinv = wp.tile([128, 1], F32, tag="ri2")
                    nc.scalar.activation(s_sb[:], psAB[:, D_OUT:D_OUT + 1],
                                         AFT.Copy, bias=EPS)
                    nc.vector.reciprocal(rinv[:], s_sb[:])
                    o = op.tile([128, D_OUT], F32, tag="of")
                    nc.scalar.activation(o[:], psAB[:, 0:D_OUT], AFT.Copy,
                                         scale=rinv[:])
                    nc.sync.dma_start(out_d[t * 128:(t + 1) * 128, :], o[:])

        l2_phase()

    nc.compile()
    return nc


# ---------------------------------------------------------------- entry point

def make_in_maps(cfg, host):
    return [{
        "xT": host["xT"][c], "W1e": host["W1e"], "W2e": host["W2e"],
        "idx": host["idx"][c], "oh": host["oh"][c], "ohT": host["ohT"][c],
        "ident": host["ident"],
    } for c in range(cfg["cores"])]


def kernel(x, edge_index, W1, a1_src, a1_dst, b1, W2, a2_src, a2_dst, b2,
           cores=8, runner=None):
    x = np.asarray(x)
    edge_index = np.asarray(edge_index)
    assert np.allclose(np.asarray(b1), 0.0), "nonzero b1 unsupported"
    agspec = os.environ.get("GAT_AGCH", "6,8,8,8")
    ag_g = [int(v) for v in agspec.split(",") if v] if agspec != "0" else 0
    cfg, host, post = prep(x, edge_index, W1, a1_src, a1_dst,
                           W2, a2_src, a2_dst, cores, ag_g)
    nc = build(cfg)
    in_maps = make_in_maps(cfg, host)
    if runner is None:
        from concourse.bass_utils import run_bass_kernel_spmd
        res = run_bass_kernel_spmd(nc, in_maps, core_ids=list(range(cores)),
                                   trace=os.environ.get("GAT_TRACE", "") == "1")
        outs = [r["out"] for r in res.results]
        kernel.last_exec_ns = res.exec_time_ns
    else:
        outs = runner(nc, in_maps)
    slots = np.concatenate(outs, 0)          # [cores*S, D_OUT]
    full = slots[post["slot"]]               # back to node order
    return (full + np.asarray(b2)[None, :]).astype(np.float32)


kernel.last_exec_ns = None
